# revision 2
# baseline (speedup 1.0000x reference)
"""Trainium2 Bass kernel for nn_CalWeight: per-row atan2 angles + circular diff.

Reference (row-wise independent over B=16384 rows):
    col = x[:, 0:1]; row = x[:, 1:2]; verts = x[:, 2:].reshape(B, N, 2)
    phi  = arctan2(verts[..., 1] - row, verts[..., 0] - col)     # [B, N]
    out  = phi - roll(phi, -1, axis=1)                           # [B, N]

Sharding: B across 8 NeuronCores (data parallel, no comms); 128-row tiles.

Strategy (DMA-bound problem: 16.8 MB in + 8.4 MB out per core ~ 76 us at
332 GB/s). All elementwise compute is kept under the input-DMA stream time
by running the DVE passes in fp16 (packed 2-byte operands hit the DVE
4x perf mode on InstTensorScalarPtr) and spreading work across ACT + Pool:

    rt  = 1/(col - vx) = -1/dx     (ACT Reciprocal table, scale=-1 bias=col,
                                    fp16 out; ~1e-5 table err + 5e-4 round)
    dyh = vy - row                 (Pool tensor_scalar, fp16 out)
    q'  = dyh * rt = -q            (DVE STT fp16 4x)
    q'  = clamp(q', +-60000)       (DVE TS fp16 4x; kills the ~1e-5-rate
                                    inf from fp16 |rt| overflow; atan(6e4)
                                    is pi/2 to 1.7e-5)
    hdy = [dyh >= 0]               (DVE TS fp16 4x)
    u8  = [q' <= 0] - hdy          (DVE STT fp16 4x)
    tp  = atan(q')                 (ACT Arctan table, fp16)
    PHI = tp + pi*u8  == -phi + C  (DVE STT fp16 4x, in place)
    out[j] = phi[j] - phi[j+1] = PHI[j+1] - PHI[j]
                                   (DVE TT, f32 out at full rate; plus a
                                    [P,1] wrap op)

The quadrant identity phi = atan(q) + pi*[dy>=0] - pi*[q>=0] is exact;
comparators (not Sign) keep the dy == +0 (dx > 0) samples correct.
fp16 end-to-end rel err is ~3e-4 (simulated on the real distribution).

ACT Reciprocal and Arctan live in different activation-table sets
(Identity/Copy are in every set), so tiles are processed in groups of
GROUP: recip-table pass over the group, then trig-table pass -> 4 table
loads total (1283 ns each). Group g's output DMAs overlap group g+1's
input DMAs, and output data is ready the moment the input stream ends,
so the wall clock rides the ~76 us DMA roofline instead of serializing
read phase then write phase.
"""

import numpy as np

import concourse.bass as bass
import concourse.bacc as bacc
import concourse.mybir as mybir
from concourse.tile import TileContext
from concourse.tile_rust import add_dep_helper

P = 128
N = 1024
COLS = 2 + 2 * N  # 2050
B_FULL = 16384
N_CORES = 8
B_SHARD = B_FULL // N_CORES  # 2048
GROUP = 8  # tiles per activation-table phase

PI = float(np.pi)
QCLAMP = 60000.0

F32 = mybir.dt.float32
F16 = mybir.dt.float16
AF = mybir.ActivationFunctionType
ALU = mybir.AluOpType

# rt = Reciprocal(-vx + col) fuses the dx subtract into the ACT pass via
# scale/bias. The activation() wrapper bans Reciprocal (and float-only bias
# for it); _act_raw emits InstActivation directly. Fallback (if the fused
# form miscompiles): set False to compute dx = vx - col on Pool and feed
# Reciprocal plain, flipping the pipeline to un-negated signs.
USE_ACT_BIAS_RECIP = True


def _act_raw(nc, out_ap, in_ap, func, bias=0.0, scale=1.0):
    """Emit InstActivation directly (bypasses the Reciprocal wrapper ban)."""
    ins = [nc.scalar.lower_ap(in_ap)]
    for arg in (bias, scale, 0.0):
        if isinstance(arg, (float, int)):
            ins.append(mybir.ImmediateValue(dtype=F32, value=float(arg)))
        else:
            ins.append(nc.scalar.lower_ap(arg))
    return nc.scalar.add_instruction(
        mybir.InstActivation(
            name=nc.get_next_instruction_name(),
            func=func,
            ins=ins,
            outs=[nc.scalar.lower_ap(out_ap)],
        )
    )


def build_nc(rows: int = B_SHARD) -> bass.Bass:
    """Build the single-core Bass program: x[rows, 2050] -> out[rows, 1024]."""
    assert rows % P == 0
    ntiles = rows // P

    nc = bacc.Bacc("TRN2", target_bir_lowering=False)
    x = nc.dram_tensor("x", [rows, COLS], F32, kind="ExternalInput")
    out = nc.dram_tensor("out", [rows, N], F32, kind="ExternalOutput")

    with TileContext(nc, pool_alloc_mode="queue") as tc:
        with (
            tc.tile_pool(name="io", bufs=5) as iop,
            tc.tile_pool(name="dx", bufs=3) as dxp,
            tc.tile_pool(name="dy", bufs=3) as dyp,
            tc.tile_pool(name="rt", bufs=3) as rtp,
            tc.tile_pool(name="qt", bufs=GROUP + 1) as qtp,
            tc.tile_pool(name="u8", bufs=GROUP + 1) as u8p,
            tc.tile_pool(name="tp", bufs=3) as tpp,
            tc.tile_pool(name="ang", bufs=4) as angp,
        ):
            keep = {}
            prev_act = None

            def chain(inst):
                nonlocal prev_act
                if prev_act is not None:
                    add_dep_helper(inst.ins, prev_act.ins, sync=False,
                                   reason="ACT table-phase ordering")
                prev_act = inst

            for g0 in range(0, ntiles, GROUP):
                tiles = range(g0, min(g0 + GROUP, ntiles))

                # ---- reciprocal-table phase ----
                for i in tiles:
                    raw = iop.tile([P, COLS], F32, tag="raw")
                    nc.sync.dma_start(out=raw[:], in_=x[i * P : (i + 1) * P, :])

                    col = raw[:, 0:1]
                    row = raw[:, 1:2]
                    vx = raw[:, 2::2]
                    vy = raw[:, 3::2]

                    rt = rtp.tile([P, N], F16, tag="rt")
                    if USE_ACT_BIAS_RECIP:
                        # rt = 1/(col - vx) = -1/dx
                        chain(_act_raw(nc, rt[:], vx, AF.Reciprocal,
                                       bias=col, scale=-1.0))
                    else:
                        dx = dxp.tile([P, N], F32, tag="dx")
                        nc.gpsimd.tensor_scalar(
                            out=dx[:], in0=vx, scalar1=col, scalar2=None,
                            op0=ALU.subtract,
                        )
                        # rt = 1/dx
                        chain(_act_raw(nc, rt[:], dx[:], AF.Reciprocal))

                    # dyh = vy - row   (fp16, on Pool)
                    dyh = dyp.tile([P, N], F16, tag="dyh")
                    nc.gpsimd.tensor_scalar(
                        out=dyh[:], in0=vy, scalar1=row, scalar2=None,
                        op0=ALU.subtract,
                    )

                    # qt = dyh * rt   (+-q), clamped to fp16-safe range
                    qt = qtp.tile([P, N], F16, tag="qt")
                    nc.vector.scalar_tensor_tensor(
                        qt[:], in0=dyh[:], scalar=1.0, in1=rt[:],
                        op0=ALU.mult, op1=ALU.mult,
                    )
                    nc.vector.tensor_scalar(
                        out=qt[:], in0=qt[:], scalar1=QCLAMP, scalar2=-QCLAMP,
                        op0=ALU.min, op1=ALU.max,
                    )
                    # hdy = [dyh >= 0]
                    hdy = dyp.tile([P, N], F16, tag="hdy")
                    nc.vector.tensor_scalar(
                        out=hdy[:], in0=dyh[:], scalar1=0.0, scalar2=None,
                        op0=ALU.is_ge,
                    )
                    # u8 = [qt <= 0] - hdy  (negated sign) or [qt >= 0] - hdy
                    u8 = u8p.tile([P, N], F16, tag="u8")
                    nc.vector.scalar_tensor_tensor(
                        u8[:], in0=qt[:], scalar=0.0, in1=hdy[:],
                        op0=(ALU.is_le if USE_ACT_BIAS_RECIP else ALU.is_ge),
                        op1=ALU.subtract,
                    )
                    keep[i] = (qt, u8)

                # ---- trig-table phase + assembly + store ----
                for i in tiles:
                    qt, u8 = keep.pop(i)
                    tp = tpp.tile([P, N], F16, tag="tp")
                    chain(nc.scalar.activation(tp[:], qt[:], AF.Arctan))
                    # PHI = tp +- pi*u8   (in place)
                    nc.vector.scalar_tensor_tensor(
                        tp[:], in0=u8[:],
                        scalar=(PI if USE_ACT_BIAS_RECIP else -PI),
                        in1=tp[:], op0=ALU.mult, op1=ALU.add,
                    )
                    ang = angp.tile([P, N], F32, tag="ang")
                    if USE_ACT_BIAS_RECIP:
                        # out[j] = PHI[j+1] - PHI[j]; wrap: PHI[0] - PHI[N-1]
                        nc.vector.tensor_tensor(
                            out=ang[:, 0 : N - 1], in0=tp[:, 1:N],
                            in1=tp[:, 0 : N - 1], op=ALU.subtract,
                        )
                        nc.vector.tensor_tensor(
                            out=ang[:, N - 1 : N], in0=tp[:, 0:1],
                            in1=tp[:, N - 1 : N], op=ALU.subtract,
                        )
                    else:
                        # out[j] = phi[j] - phi[j+1]; wrap: phi[N-1] - phi[0]
                        nc.vector.tensor_tensor(
                            out=ang[:, 0 : N - 1], in0=tp[:, 0 : N - 1],
                            in1=tp[:, 1:N], op=ALU.subtract,
                        )
                        nc.vector.tensor_tensor(
                            out=ang[:, N - 1 : N], in0=tp[:, N - 1 : N],
                            in1=tp[:, 0:1], op=ALU.subtract,
                        )
                    nc.sync.dma_start(out=out[i * P : (i + 1) * P, :], in_=ang[:])

    nc.compile()
    return nc


_NC_CACHE = {}


def _get_nc(rows: int) -> bass.Bass:
    if rows not in _NC_CACHE:
        _NC_CACHE[rows] = build_nc(rows)
    return _NC_CACHE[rows]


def run_sharded(x: np.ndarray, **run_kwargs):
    """Shard x over 8 cores, run, return (full_output, BassKernelResults)."""
    from concourse.bass_utils import run_bass_kernel_spmd

    x = np.ascontiguousarray(x, dtype=np.float32)
    assert x.shape == (B_FULL, COLS), x.shape

    nc = _get_nc(B_SHARD)
    shards = [x[i * B_SHARD : (i + 1) * B_SHARD] for i in range(N_CORES)]
    in_maps = [{"x": s} for s in shards]
    res = run_bass_kernel_spmd(nc, in_maps, core_ids=list(range(N_CORES)), **run_kwargs)
    outs = [r["out"] for r in res.results]
    return np.concatenate(outs, axis=0), res


def kernel(x: np.ndarray) -> np.ndarray:
    """Full-input entry point: x [16384, 2050] f32 -> [16384, 1024] f32."""
    full, _ = run_sharded(x)
    return full


# revision 3
# speedup vs baseline: 3.0496x; 3.0496x over previous
"""Trainium2 Bass kernel for nn_CalWeight: per-row atan2 angles + circular diff.

Reference (row-wise independent over B=16384 rows):
    col = x[:, 0:1]; row = x[:, 1:2]; verts = x[:, 2:].reshape(B, N, 2)
    phi  = arctan2(verts[..., 1] - row, verts[..., 0] - col)     # [B, N]
    out  = phi - roll(phi, -1, axis=1)                           # [B, N]

Sharding: B across 8 NeuronCores (data parallel, no comms); 128-row tiles.

DMA-bound problem: 16.8 MB in + 8.4 MB out per core ~ 76 us at ~332 GB/s.
The job of the compute pipeline is to stay under that. Engine notes from
measurement: Pool tensor ops are Q7-software (~15 us per [128,1024] op --
unusable); DVE fp16 is a slow path (4x WORSE than f32); DVE bf16 packed
SBUF operands hit the documented fast modes (tensor_scalar 4x,
tensor_tensor / scalar_tensor_tensor 2x). So: bf16 everywhere on DVE,
nothing on Pool, and the two strided-f32 passes on ACT.

    rt  = 1/(col - vx) = -1/dx     (ACT Reciprocal table, scale=-1 bias=col
                                    fused, bf16 out)
    ndy = row - vy = -dy           (ACT Identity, scale=-1 bias=row, bf16;
                                    Identity is in every table set)
    q   = ndy * rt = dy/dx         (DVE STT bf16 2x)
    q   = clamp(q, +-1e30)         (DVE TS bf16 4x; if dx == +-0.0 exactly,
                                    rt and q go inf and the Arctan table's
                                    behavior on inf is undefined; atan(1e30)
                                    is pi/2 exactly in bf16)
    hdy = [ndy <= 0] == [dy >= 0]  (DVE TS bf16 4x)
    u8  = [q >= 0] - hdy           (DVE STT bf16 2x)
    tp  = atan(q)                  (ACT Arctan table, bf16)
    phi = tp - pi*u8               (DVE STT bf16 2x, in place; exact
                                    quadrant identity
                                    phi = atan(q) + pi*[dy>=0] - pi*[q>=0],
                                    comparators keep dy == +0, dx > 0 right)
    out[j] = phi[j] - phi[j+1]     (DVE TT, f32 out at full rate; [P,1]
                                    wrap op for j = N-1)

bf16 end-to-end rel err ~2e-3 (simulated on the real distribution;
harness gate is 2e-2).

ACT Reciprocal and Arctan live in different activation-table sets, so
tiles run in groups of GROUP: recip-table pass over the group, then
trig-table pass -> 4 table loads total (1283 ns each). Group g's output
DMAs queue behind group g+1's input DMAs and stream the moment the input
stream ends, so the wall clock rides the ~76 us DMA roofline instead of
serializing a read phase then a write phase.
"""

import numpy as np

import concourse.bass as bass
import concourse.bacc as bacc
import concourse.mybir as mybir
from concourse.tile import TileContext
from concourse.tile_rust import add_dep_helper

P = 128
N = 1024
COLS = 2 + 2 * N  # 2050
B_FULL = 16384
N_CORES = 8
B_SHARD = B_FULL // N_CORES  # 2048
GROUP = 8  # tiles per activation-table phase

PI = float(np.pi)
QCLAMP = 1e30

F32 = mybir.dt.float32
BF16 = mybir.dt.bfloat16
AF = mybir.ActivationFunctionType
ALU = mybir.AluOpType


def _act_raw(nc, out_ap, in_ap, func, bias=0.0, scale=1.0):
    """Emit InstActivation directly (bypasses the Reciprocal wrapper ban)."""
    ins = [nc.scalar.lower_ap(in_ap)]
    for arg in (bias, scale, 0.0):
        if isinstance(arg, (float, int)):
            ins.append(mybir.ImmediateValue(dtype=F32, value=float(arg)))
        else:
            ins.append(nc.scalar.lower_ap(arg))
    return nc.scalar.add_instruction(
        mybir.InstActivation(
            name=nc.get_next_instruction_name(),
            func=func,
            ins=ins,
            outs=[nc.scalar.lower_ap(out_ap)],
        )
    )


def build_nc(rows: int = B_SHARD) -> bass.Bass:
    """Build the single-core Bass program: x[rows, 2050] -> out[rows, 1024]."""
    assert rows % P == 0
    ntiles = rows // P

    nc = bacc.Bacc("TRN2", target_bir_lowering=False)
    x = nc.dram_tensor("x", [rows, COLS], F32, kind="ExternalInput")
    out = nc.dram_tensor("out", [rows, N], F32, kind="ExternalOutput")

    with TileContext(nc, pool_alloc_mode="queue") as tc:
        with (
            tc.tile_pool(name="io", bufs=5) as iop,
            tc.tile_pool(name="nd", bufs=3) as ndp,
            tc.tile_pool(name="rt", bufs=3) as rtp,
            tc.tile_pool(name="qt", bufs=GROUP + 1) as qtp,
            tc.tile_pool(name="u8", bufs=GROUP + 1) as u8p,
            tc.tile_pool(name="tp", bufs=3) as tpp,
            tc.tile_pool(name="ang", bufs=4) as angp,
        ):
            keep = {}
            prev_act = None

            def chain(inst):
                nonlocal prev_act
                if prev_act is not None:
                    add_dep_helper(inst.ins, prev_act.ins, sync=False,
                                   reason="ACT table-phase ordering")
                prev_act = inst

            for g0 in range(0, ntiles, GROUP):
                tiles = range(g0, min(g0 + GROUP, ntiles))

                # ---- reciprocal-table phase ----
                for i in tiles:
                    raw = iop.tile([P, COLS], F32, tag="raw")
                    nc.sync.dma_start(out=raw[:], in_=x[i * P : (i + 1) * P, :])

                    col = raw[:, 0:1]
                    row = raw[:, 1:2]
                    vx = raw[:, 2::2]
                    vy = raw[:, 3::2]

                    # rt = 1/(col - vx) = -1/dx
                    rt = rtp.tile([P, N], BF16, tag="rt")
                    chain(_act_raw(nc, rt[:], vx, AF.Reciprocal,
                                   bias=col, scale=-1.0))
                    # ndy = row - vy = -dy  (Identity is in every table set)
                    ndy = ndp.tile([P, N], BF16, tag="ndy")
                    chain(nc.scalar.activation(ndy[:], vy, AF.Identity,
                                               bias=row, scale=-1.0))

                    # q = ndy * rt = dy/dx, clamped finite
                    qt = qtp.tile([P, N], BF16, tag="qt")
                    nc.vector.scalar_tensor_tensor(
                        qt[:], in0=ndy[:], scalar=1.0, in1=rt[:],
                        op0=ALU.mult, op1=ALU.mult,
                    )
                    nc.vector.tensor_scalar(
                        out=qt[:], in0=qt[:], scalar1=QCLAMP, scalar2=-QCLAMP,
                        op0=ALU.min, op1=ALU.max,
                    )
                    # hdy = [ndy <= 0] = [dy >= 0]
                    hdy = ndp.tile([P, N], BF16, tag="hdy")
                    nc.vector.tensor_scalar(
                        out=hdy[:], in0=ndy[:], scalar1=0.0, scalar2=None,
                        op0=ALU.is_le,
                    )
                    # u8 = [q >= 0] - hdy
                    u8 = u8p.tile([P, N], BF16, tag="u8")
                    nc.vector.scalar_tensor_tensor(
                        u8[:], in0=qt[:], scalar=0.0, in1=hdy[:],
                        op0=ALU.is_ge, op1=ALU.subtract,
                    )
                    keep[i] = (qt, u8)

                # ---- trig-table phase + assembly + store ----
                for i in tiles:
                    qt, u8 = keep.pop(i)
                    tp = tpp.tile([P, N], BF16, tag="tp")
                    chain(nc.scalar.activation(tp[:], qt[:], AF.Arctan))
                    # phi = tp - pi*u8   (in place)
                    nc.vector.scalar_tensor_tensor(
                        tp[:], in0=u8[:], scalar=-PI, in1=tp[:],
                        op0=ALU.mult, op1=ALU.add,
                    )
                    # out[j] = phi[j] - phi[j+1]; wrap: phi[N-1] - phi[0]
                    ang = angp.tile([P, N], F32, tag="ang")
                    nc.vector.tensor_tensor(
                        out=ang[:, 0 : N - 1], in0=tp[:, 0 : N - 1],
                        in1=tp[:, 1:N], op=ALU.subtract,
                    )
                    nc.vector.tensor_tensor(
                        out=ang[:, N - 1 : N], in0=tp[:, N - 1 : N],
                        in1=tp[:, 0:1], op=ALU.subtract,
                    )
                    nc.sync.dma_start(out=out[i * P : (i + 1) * P, :], in_=ang[:])

    nc.compile()
    return nc


_NC_CACHE = {}


def _get_nc(rows: int) -> bass.Bass:
    if rows not in _NC_CACHE:
        _NC_CACHE[rows] = build_nc(rows)
    return _NC_CACHE[rows]


def run_sharded(x: np.ndarray, **run_kwargs):
    """Shard x over 8 cores, run, return (full_output, BassKernelResults)."""
    from concourse.bass_utils import run_bass_kernel_spmd

    x = np.ascontiguousarray(x, dtype=np.float32)
    assert x.shape == (B_FULL, COLS), x.shape

    nc = _get_nc(B_SHARD)
    shards = [x[i * B_SHARD : (i + 1) * B_SHARD] for i in range(N_CORES)]
    in_maps = [{"x": s} for s in shards]
    res = run_bass_kernel_spmd(nc, in_maps, core_ids=list(range(N_CORES)), **run_kwargs)
    outs = [r["out"] for r in res.results]
    return np.concatenate(outs, axis=0), res


def kernel(x: np.ndarray) -> np.ndarray:
    """Full-input entry point: x [16384, 2050] f32 -> [16384, 1024] f32."""
    full, _ = run_sharded(x)
    return full
